# revision 51
# baseline (speedup 1.0000x reference)
"""Channel-attention (CAM) Trainium2 kernel, v8 (final).

Reference computation (per batch b of 16):
    q   = x[b].reshape(C, HW)                  # C=512, HW=4096
    sim = q @ q.T                              # [C, C], symmetric
    attn = softmax(max(sim) - sim, axis=-1)    # == exp(min_r - sim) / Z_r
    out[b] = gamma * attn @ x[b] + x[b]

Sharding: data-parallel over batch across 8 NeuronCores (2 batches/core).
kernel() takes full inputs, shards internally, returns the full output.
Measured ~100us/core vs the 118us fp16 v7 baseline.

Design (on top of v7's interleaved-emission skeleton):

  mm1  sim upper block-triangle in fp16 (sim feeds an exp, so it needs
       fp16 precision; fp8-with-correction needs 3 half-rate products vs
       1 full-rate, strictly worse). Lower blocks mirrored by PE
       transpose (regular matmuls against identity — keeps the HAM
       clock gate seeing a busy PE).
  mm2  computed TRANSPOSED in fp8 DoubleRow (2x PE throughput):
           outT[hw, c] = qT @ (gamma*attn^T/Z) + qT
       stationary = qn8 (host-cast fp8 q, channel-pair layout
       [t2, P, 2, hw]), moving = attnT8 (PE-transposed softmax rows,
       cast fp32->fp8 on the PSUM->SBUF copyback). 2 DoubleRow matmuls
       per 128x512 group instead of 4 fp16 ones. Emitted in PAIRS of hw
       blocks ([P, 2, C] 2-bank PSUM tiles) so copy/add instruction
       overheads amortize over 1024 elements.
  residual  the +x term must NOT pass through fp8 (3% noise on x blows
       the error budget), so it is NOT identity-folded in fp8: the fp16
       qt tile (loaded for mm1 anyway) is added on the way out, via a
       per-pair route mix (ACT copy + DVE/GPSIMD add, fused DVE
       PSUM-add, or a PE fp16-identity matmul) chosen so every engine
       stays under the PE pair rate in each phase. Output is fp16 outT
       in pair-blocked layout; the host re-indexes and casts fp32.
       Removes the qn fp16 load entirely (DMA 21MB/core vs v7 33.6MB).

Scheduling (PE is in-order; emission order = schedule):
  - warmups read a memset tile (no DMA dependency) so the PE is busy
    from the end of the ~7us engine preamble and the p-state ramp is
    done before mm1; sized to bridge until qt0's transfer lands.
  - mm1(b1) is row-major filler inside softmax(0); rows open in pairs
    {0,1}/{2,3} gated on the odd exp of batch 0, because mm1 rows live
    in HALVES of [P, 2, C] psim pair-buffers — which later serve as
    full 2-bank mm2 pairs, giving the drain a 4-pair PSUM rotation.
  - mm2(b0) pairs are filler inside softmax(1); mm2(b1) drains after.
  - transposed-p accumulators (pst) also live as pair-buffer halves and
    take transpose blocks per-ci right after each p_s, so only one
    ci-column + the fp8 casts (split ACT/DVE) remain after the last p_s.
  - stores: one 256KB DMA per pair, issued with a 2-pair lag (an ACT/
    sync-queue dma_start must never wait on an unfinished add), on the
    sync + scalar HARDWARE DGE rings only; gpsimd's ring is software
    DGE (~2x slower — it caused a 5us end-of-kernel drain and an
    intermittent corruption when mixed with GPSIMD compute).

  Measured numerics (host sim matches hw exactly): rel_l2 1.15e-2 vs
  the 2e-2 gate (fp8 rounding of attn and q contribute ~0.8e-2 each).
"""
import sys

if "/opt/trn_rl_repo" not in sys.path:
    sys.path.insert(0, "/opt/trn_rl_repo")

import numpy as np
import ml_dtypes

B, C, H, W = 16, 512, 64, 64
HW = H * W
NCORES = 8
NB = B // NCORES          # batches per core
P = 128
CB = C // P               # 4 channel blocks
KN = HW // P              # 32 contraction chunks for sim
KT = KN // 4              # 8 transposed-q tiles of 4 chunks each
T2 = C // 256             # 2 channel double-tiles for DoubleRow mm2
NPAIR = HW // P // 2      # 16 mm2 output pairs (of 128-row hw blocks)

_BUILD_CACHE = {}


def build_bass():
    import concourse.bacc as bacc
    import concourse.tile as tile
    from concourse import mybir

    f32 = mybir.dt.float32
    f16 = mybir.dt.float16
    f8 = mybir.dt.float8e4
    AX = mybir.AxisListType
    ALU = mybir.AluOpType
    ACTF = mybir.ActivationFunctionType
    DR = mybir.MatmulPerfMode.DoubleRow

    nc = bacc.Bacc()
    # qt: q transposed, host-arranged as [KT, P, 4, C] so one DMA per
    # 4-chunk tile is fully contiguous (4KB descriptor lines). Feeds mm1
    # AND is the fp16 residual source for the transposed mm2.
    qt_ext = nc.declare_dram_parameter("qt", [NB, KT, P, 4, C], f16,
                                       isOutput=False)
    # qn8: fp8 q in channel-pair layout for DoubleRow stationary tiles:
    # qn8[b, t2, p, i, hw] = fp8(q[ch = t2*256 + i*128 + p, hw])
    q8_ext = nc.declare_dram_parameter("qn8", [NB, T2, P, 2, HW], f8,
                                       isOutput=False)
    g_ext = nc.declare_dram_parameter("gamma", [1], f32, isOutput=False)
    i_ext = nc.declare_dram_parameter("ident", [P, P], f16, isOutput=False)
    # outT stored in pair-blocked layout [b, g, p, j, c] = out^T row
    # hw=(2g+j)*128+p: one 256KB DMA per pair with contiguous 2KB runs;
    # the host re-indexes (transpose(0,1,3,2,4)) for free
    o_ext = nc.declare_dram_parameter("outT", [NB, NPAIR, P, 2, C], f16,
                                      isOutput=True)

    _flip = [0]

    with tile.TileContext(nc) as tc:
        with (
            tc.tile_pool(name="const", bufs=1) as const,
            tc.tile_pool(name="qt", bufs=16) as qtp,
            tc.tile_pool(name="qn8", bufs=4) as qn8p,
            tc.tile_pool(name="a8", bufs=4) as a8p,
            tc.tile_pool(name="pp", bufs=4) as pp,
            tc.tile_pool(name="fsb", bufs=8) as fsbp,
            tc.tile_pool(name="stg", bufs=10) as stgp,
            tc.tile_pool(name="tri", bufs=6) as trip,
            tc.tile_pool(name="vec", bufs=6) as vec,
            tc.tile_pool(name="psim", bufs=2, space="PSUM") as psimp,
            tc.tile_pool(name="ppair", bufs=2, space="PSUM") as ppairp,
        ):
            def copyback(dst, src):
                if _flip[0] % 2 == 0:
                    nc.scalar.copy(dst, src)
                else:
                    nc.vector.tensor_copy(dst, src)
                _flip[0] += 1

            # ident+gamma on the scalar HW ring so the sync ring's first
            # transfer is the critical qt0 tile
            ident_h = const.tile([P, P], f16)
            nc.scalar.dma_start(out=ident_h[:], in_=i_ext[:])
            gamma_sb = const.tile([P, 1], f32)
            nc.scalar.dma_start(out=gamma_sb[:], in_=g_ext[:].to_broadcast([P, 1]))

            # data loads all on the sync ring in need-order (one ring's
            # queued transfers already saturate HBM; the first-transfer
            # delay is completion latency, not bandwidth — splitting
            # loads across rings only delays the first tile)
            def load_qt(b, st, t):
                qt4 = qtp.tile([P, 4, C], f16, tag="qt", name=f"qt{b}_{t}")
                nc.sync.dma_start(out=qt4[:, :, :], in_=qt_ext[b, t])
                st["qt"][t] = qt4

            def load_qn8(b, st, t2):
                q8 = qn8p.tile([P, 2, HW], f8, tag="qn8", name=f"q8{b}_{t2}")
                nc.sync.dma_start(out=q8[:, :, :], in_=q8_ext[b, t2])
                st["qn8"][t2] = q8

            def alloc_state(b):
                return {"qt": {}, "qn8": {}, "psim": {}, "a8": {}}

            def mm_transpose(out, in_):
                nc.tensor.matmul(out, in_, ident_h[:], start=True, stop=True)

            # real warmup matmuls while the first loads land (HAM warm-up;
            # sized to end right as qt0's transfer completes, so the
            # p-state ramp doesn't reset and mm1 starts at full clock).
            # They read an UNINITIALIZED SBUF tile — no DMA dependency, so
            # they start at ~6us (right after the engine preamble) instead
            # of waiting ~4us for ident's first-DMA completion latency;
            # the garbage results land in a psum tile nobody reads.
            warm_src = const.tile([P, P], f16)
            nc.gpsimd.memset(warm_src[:], 0)
            warm = ppairp.tile([P, 2, C], f32, tag="pp", name="warmup")
            for i in range(64):
                nc.tensor.matmul(warm[:, 0, :P], warm_src[:], warm_src[:],
                                 start=True, stop=True)

            C0S = [mi * P for mi in range(CB)]  # exact upper triangle
            TRI = {1: [(1, 0)], 2: [(2, 0), (2, 1)], 3: [(3, 0), (3, 1), (3, 2)]}

            def alloc_psim_pair(st, mi, b):
                """mm1 rows live in halves of [P, 2, C] pair-buffers so the
                mm2 drain can reuse the psim ring as full 2-bank pairs."""
                buf = psimp.tile([P, 2, C], f32, tag="psim",
                                 name=f"psim{b}_{mi}")
                st["psim"][mi] = buf[:, 0, :]
                st["psim"][mi + 1] = buf[:, 1, :]

            def mm1_group(st, mi, kn, b):
                t, kq = divmod(kn, 4)
                c0 = C0S[mi]
                qt4 = st["qt"][t]
                nc.tensor.matmul(
                    st["psim"][mi][:, c0:],
                    qt4[:, kq, mi * P:(mi + 1) * P],
                    qt4[:, kq, c0:],
                    start=(kn == 0),
                    stop=(kn == KN - 1),
                )

            def phase1_b0(st):
                """batch 0: kn-major (starts on the first loaded tile),
                but the LAST tile's groups go row-major: row 0 stops ~2us
                before row 3, so softmax(0)'s serial reduce/exp head (which
                gates the b1 filler) overlaps real mm1 work instead of
                relying purely on dummy matmuls."""
                for mi in range(0, CB, 2):
                    alloc_psim_pair(st, mi, 0)
                for kn in range(KN - 4):
                    for mi in range(CB):
                        mm1_group(st, mi, kn, 0)
                for mi in range(CB):
                    for kn in range(KN - 4, KN):
                        mm1_group(st, mi, kn, 0)

            def mm1_b1_filler(st):
                """batch 1: row-major generator — rows open in pairs {0,1}
                then {2,3}, enabled by the caller right after the odd exp
                of batch 0 frees that psim pair-buffer."""
                state = {"mi": 0, "kn": 0, "maxmi": 0}

                def allow(mi):
                    state["maxmi"] = max(state["maxmi"], mi + 1)

                def emit(n):
                    for _ in range(n):
                        mi, kn = state["mi"], state["kn"]
                        if mi >= CB:
                            return
                        if mi >= state["maxmi"]:
                            return
                        if kn == 0 and mi % 2 == 0:
                            alloc_psim_pair(st, mi, 1)
                        mm1_group(st, mi, kn, 1)
                        if kn == KN - 1:
                            state["mi"], state["kn"] = mi + 1, 0
                        else:
                            state["kn"] = kn + 1

                def flush():
                    state["maxmi"] = CB
                    emit(CB * KN)

                return allow, emit, flush

            def softmax_a8(b, st, on_exp=None, filler=None, head_dummy=False,
                           pst_pool_tag=None):
                """tri fills + rowwise softmax (pipelined per block-row),
                then attnT8 = fp8(T(p*gamma/Z)) in channel-pair layout.
                `filler(n)` emits ready next-phase matmuls between stages
                (PE is in-order); `on_exp(mi)` notifies psim[mi] consumed.
                pst transposes borrow whichever PSUM ring is idle during
                this batch's softmax (ppair in S0, psim in S1)."""
                psim = st["psim"]
                pst_pool, pst_tag = pst_pool_tag
                # cover the serial tri-cast/reduce/exp head: real filler
                # groups if available, else dummy matmuls on loaded data
                if filler is not None and not head_dummy:
                    filler(4)
                # row 0 needs no tri mirror: launch its reduce+exp first so
                # the ACT/DVE pipelines fill immediately
                def reduce_exp(mi, fill=True):
                    mrow = vec.tile([P, 1], f32, tag="mrow")
                    nc.vector.tensor_reduce(
                        mrow[:], psim[mi][:], axis=AX.X, op=ALU.min
                    )
                    zrow = vec.tile([P, 1], f32, tag="zrow")
                    p_t = pp.tile([P, C], f16, tag="p", bufs=4)
                    nc.scalar.activation(
                        p_t[:], psim[mi][:], ACTF.Exp,
                        bias=mrow[:], scale=-1.0, accum_out=zrow[:],
                    )
                    zrows.append(zrow)
                    p_ts.append(p_t)
                    if on_exp is not None:
                        on_exp(mi)
                    if fill and filler is not None:
                        filler(5)

                zrows, p_ts = [], []
                reduce_exp(0)
                # row-1's mirror chain BEFORE the dummies, its copyback
                # forced onto DVE: the exp(1) gate (reduce(1) needs the
                # T(1,0) mirror) then completes DURING the dummy bridge
                # instead of serially after it — that serial wait showed
                # as a constant ~1.6us PE gap in every trace
                tmp10 = trip.tile([P, P], f16, tag="tri")
                nc.vector.tensor_copy(tmp10[:], psim[0][:, 1 * P:2 * P])
                mm_transpose(psim[1][:, 0:P], tmp10[:])
                if head_dummy:
                    dmy = ppairp.tile([P, 2, C], f32, tag="pp", name=f"dmy{b}")
                    src = st["qt"][KT - 1]
                    for i in range(12):
                        nc.tensor.matmul(dmy[:, 0, :], ident_h[:],
                                         src[:, i % 4, :],
                                         start=True, stop=True)
                reduce_exp(1, fill=False)
                # remaining tri tmp copybacks (the psim[j] READERS) must
                # all be emitted before the first post-allow filler call,
                # else the psim pool rotation races the unemitted reads
                tmps = {}
                for (i, j) in [(2, 0), (2, 1), (3, 0), (3, 1), (3, 2)]:
                    tmp = trip.tile([P, P], f16, tag="tri")
                    copyback(tmp[:], psim[j][:, i * P:(i + 1) * P])
                    tmps[(i, j)] = tmp
                if filler is not None:
                    filler(8)
                for mi in range(2, CB):
                    for (i, j) in TRI.get(mi, []):
                        mm_transpose(psim[i][:, j * P:(j + 1) * P],
                                     tmps[(i, j)][:])
                    reduce_exp(mi)
                # all 4 pst (transposed-p) accumulators live as halves of
                # two [P, 2, C] pair-buffers (4 banks), so transpose blocks
                # can be emitted per-ci RIGHT AFTER each p_s(ci) — after the
                # last p_s only one ci-column of transposes + the casts
                # remain, instead of the whole 16-transpose chain
                pstA = pst_pool.tile([P, 2, C], f32, tag=pst_tag,
                                     name=f"pstA{b}")
                pstB = pst_pool.tile([P, 2, C], f32, tag=pst_tag,
                                     name=f"pstB{b}")
                psts = [pstA[:, 0, :], pstA[:, 1, :],
                        pstB[:, 0, :], pstB[:, 1, :]]
                # stage B: per-row 1/Z, gamma fold, fp16 scale, transposes
                for mi in range(CB):
                    rz = vec.tile([P, 1], f32, tag="rz")
                    nc.vector.reciprocal(rz[:], zrows[mi][:])
                    rzg = vec.tile([P, 1], f32, tag="rzg")
                    nc.vector.tensor_mul(rzg[:], rz[:], gamma_sb[:])
                    p_s = pp.tile([P, C], f16, tag="psc", bufs=4)
                    nc.vector.tensor_scalar_mul(p_s[:], p_ts[mi][:], rzg[:])
                    if filler is not None:
                        filler(2)
                    for kd in range(CB):
                        mm_transpose(
                            psts[kd][:, mi * P:(mi + 1) * P],
                            p_s[:, kd * P:(kd + 1) * P],
                        )
                    if filler is not None:
                        filler(3)
                # cast fp8 into the channel-pair moving tiles
                # a8[t2][:, i, :], kd = 2*t2 + i; casts alternate ACT/DVE
                # so the tail runs 2 in parallel
                a8t = None
                for kd in range(CB):
                    t2, i = divmod(kd, 2)
                    if i == 0:
                        a8t = a8p.tile([P, 2, C], f8, tag="a8",
                                       name=f"a8_{b}_{t2}")
                        st["a8"][t2] = a8t
                    # no filler here: these casts gate the next mm2 phase,
                    # and filler copies/adds would queue ahead of them on
                    # the in-order ACT/DVE queues
                    if kd % 2 == 0:
                        nc.scalar.copy(a8t[:, i, :], psts[kd][:])
                    else:
                        nc.vector.tensor_copy(a8t[:, i, :], psts[kd][:])

            def mm2_emitter(b, st, rings, pools, mode):
                """outT[hw, :] = qT@attnT8*gamma/Z + qT, emitted in PAIRS of
                128-row hw blocks: one [P, 2, C] PSUM tile (2 banks) per
                pair. emit(n) is PE filler inside the next batch's softmax;
                n counts pairs.

                Residual routes per pair, mixed so every engine stays under
                the PE pair rate (DR pair = 864ns, +426ns if the residual
                is PE-identity-folded):
                  peresid: +2 fp16 identity matmuls, plain ACT copyback
                  direct:  one fused DVE add straight from PSUM (1209ns)
                  actgps:  ACT copy -> fp16 SBUF, GPSIMD add (GPSIMD's
                           compute FIFOs are free; only its DMA ring is
                           software-DGE-slow)
                  actdve:  ACT copy -> fp16 SBUF, DVE add (692ns, 2x rate)
                Stores are issued with a 2-pair lag so a store on the ACT
                queue never waits on an unfinished add, and go to the sync
                HW ring (idle in these phases); the last drain pairs use
                the scalar HW ring for a short final flush."""
                qn8_t, a8, qt_t = st["qn8"], st["a8"], st["qt"]
                state = {"g": 0, "pending": []}

                def ring_for(g):
                    if mode == "s1":
                        return nc.sync
                    return nc.sync if g < NPAIR - 2 else nc.scalar

                def issue_store(g, stg):
                    ring_for(g).dma_start(out=o_ext[b, g], in_=stg[:])

                def emit(n):
                    for _ in range(n):
                        g = state["g"]
                        if g >= NPAIR:
                            # terminal flush of the lagged stores
                            for gq, stgq in state["pending"]:
                                issue_store(gq, stgq)
                            state["pending"] = []
                            return
                        if mode == "s1":
                            r = g % 8
                            if g >= NPAIR - 3:
                                # last s1 pairs overlap the softmax tail
                                # whose casts gate the drain: keep their
                                # engine footprint off the ACT/DVE queues
                                route = "peresid"
                            else:
                                route = ("peresid" if r == 0 else
                                         "direct" if r % 2 == 1 else "actgps")
                        elif g >= NPAIR - 3:
                            # no slow GPSIMD adds near the tail
                            route = "direct" if g % 2 == 1 else "actdve"
                        else:
                            route = ("direct" if g % 2 == 1 else
                                     "actgps" if g % 4 == 0 else "actdve")
                        pool, ptag = pools[g % len(pools)]
                        pf = pool.tile([P, 2, C], f32, tag=ptag,
                                       name=f"pf{b}_{g}")
                        m0 = 2 * g
                        t, kq = divmod(m0, 4)
                        for j in range(2):
                            m = 2 * g + j
                            for t2 in range(T2):
                                nc.tensor.matmul(
                                    pf[:, j, :],
                                    qn8_t[t2][:, :, m * P:(m + 1) * P],
                                    a8[t2][:, :, :],
                                    start=(t2 == 0),
                                    stop=(t2 == T2 - 1 and route != "peresid"),
                                    perf_mode=DR,
                                )
                            if route == "peresid":
                                # residual: out^T block += I @ qT chunk
                                nc.tensor.matmul(
                                    pf[:, j, :],
                                    ident_h[:],
                                    qt_t[t][:, kq + j, :],
                                    start=False,
                                    stop=True,
                                )
                        stg = stgp.tile([P, 2, C], f16, tag="stg")
                        if route == "peresid":
                            if g % 2 == 0:
                                nc.scalar.copy(stg[:], pf[:])
                            else:
                                nc.vector.tensor_copy(stg[:], pf[:])
                        elif route == "direct":
                            nc.vector.tensor_add(stg[:], pf[:],
                                                 qt_t[t][:, kq:kq + 2, :])
                        else:
                            fsb = fsbp.tile([P, 2, C], f16, tag="fsb")
                            nc.scalar.copy(fsb[:], pf[:])
                            eng = nc.gpsimd if route == "actgps" else nc.vector
                            eng.tensor_add(stg[:], fsb[:],
                                           qt_t[t][:, kq:kq + 2, :])
                        state["pending"].append((g, stg))
                        if len(state["pending"]) > 2:
                            gq, stgq = state["pending"].pop(0)
                            issue_store(gq, stgq)
                        state["g"] = g + 1

                return emit

            # ---- emission ----
            # keep the number of in-flight DMAs at startup small (8 DMAHW
            # semaphore lanes): ident + qt(b0) first, everything else after
            st0 = alloc_state(0)
            st1 = alloc_state(1)
            for t in range(KT):
                load_qt(0, st0, t)
            phase1_b0(st0)
            # need-order on the single load ring: qt1 feeds the mm1(1)
            # filler; qn8 is only needed by mm2
            for t in range(KT):
                load_qt(1, st1, t)
            for t2 in range(T2):
                load_qn8(0, st0, t2)
            for t2 in range(T2):
                load_qn8(1, st1, t2)
            allow, emit_mm1, flush_mm1 = mm1_b1_filler(st1)

            def sm0_on_exp(mi):
                # b1's psim pair-buffer {mi-1, mi} only frees at the odd exp
                if mi % 2 == 1:
                    allow(mi)

            softmax_a8(0, st0, on_exp=sm0_on_exp, filler=emit_mm1,
                       head_dummy=True, pst_pool_tag=(ppairp, "pp"))
            flush_mm1()
            rings = [nc.sync, nc.scalar]
            mm2_0 = mm2_emitter(0, st0, rings, pools=[(ppairp, "pp")],
                                mode="s1")
            softmax_a8(1, st1, filler=mm2_0, pst_pool_tag=(psimp, "psim"))
            mm2_0(NPAIR + 1)  # flush remaining pairs + lagged stores
            # drain: psim ring is free now — rotate pairs across both rings
            # (4 pairs in flight) so PE never waits on a copyback
            mm2_1 = mm2_emitter(1, st1, rings,
                                pools=[(ppairp, "pp"), (psimp, "psim")],
                                mode="mix")
            mm2_1(NPAIR + 1)

    nc.finalize()
    return nc


def get_bass():
    if "nc" not in _BUILD_CACHE:
        _BUILD_CACHE["nc"] = build_bass()
    return _BUILD_CACHE["nc"]


_IDENT = None


def make_in_maps(x, gamma):
    global _IDENT
    if _IDENT is None:
        _IDENT = np.eye(P, dtype=np.float16)
    x = np.asarray(x, dtype=np.float32).reshape(B, C, HW)
    qn16 = x.astype(np.float16)
    # [B, KT, P, 4, C]: within each 4-chunk tile, partition-major so the
    # device DMA is fully contiguous (4KB descriptor lines)
    qt = np.ascontiguousarray(
        qn16.transpose(0, 2, 1)          # [B, HW, C]
        .reshape(B, KT, 4, P, C)         # [B, t, k, p, C]
        .transpose(0, 1, 3, 2, 4)        # [B, t, p, k, C]
    )
    # [B, T2, P, 2, HW] fp8 channel-pair layout (cast from fp32 directly)
    qn8 = np.ascontiguousarray(
        x.reshape(B, T2, 2, P, HW)       # [B, t2, i, p, hw]
        .transpose(0, 1, 3, 2, 4)        # [B, t2, p, i, hw]
    ).astype(ml_dtypes.float8_e4m3)
    gamma = np.asarray(gamma, dtype=np.float32).reshape(1)
    return [
        {
            "qt": qt[i * NB:(i + 1) * NB],
            "qn8": qn8[i * NB:(i + 1) * NB],
            "gamma": gamma,
            "ident": _IDENT,
        }
        for i in range(NCORES)
    ]


def run(x, gamma, trace=False, **trace_kwargs):
    from concourse.bass_utils import run_bass_kernel_spmd

    nc = get_bass()
    res = run_bass_kernel_spmd(
        nc, make_in_maps(x, gamma), core_ids=list(range(NCORES)),
        trace=trace, **trace_kwargs,
    )
    outT = np.concatenate(
        [res.results[i]["outT"] for i in range(NCORES)], axis=0
    )  # [B, NPAIR, P, 2, C] fp16, pair-blocked: row hw = (2g+j)*128+p
    outT = outT.transpose(0, 1, 3, 2, 4).reshape(B, HW, C)
    out = outT.astype(np.float32).transpose(0, 2, 1).reshape(B, C, H, W)
    return np.ascontiguousarray(out), res


def kernel(x, gamma):
    out, _ = run(x, gamma, trace=False)
    return out


# revision 52
# speedup vs baseline: 1.2562x; 1.2562x over previous
"""Channel-attention (CAM) Trainium2 kernel, v8 (final).

Reference computation (per batch b of 16):
    q   = x[b].reshape(C, HW)                  # C=512, HW=4096
    sim = q @ q.T                              # [C, C], symmetric
    attn = softmax(max(sim) - sim, axis=-1)    # == exp(min_r - sim) / Z_r
    out[b] = gamma * attn @ x[b] + x[b]

Sharding: data-parallel over batch across 8 NeuronCores (2 batches/core).
kernel() takes full inputs, shards internally, returns the full output.
Measured ~100us/core vs the 118us fp16 v7 baseline.

Design (on top of v7's interleaved-emission skeleton):

  mm1  sim upper block-triangle in fp16 (sim feeds an exp, so it needs
       fp16 precision; fp8-with-correction needs 3 half-rate products vs
       1 full-rate, strictly worse). Lower blocks mirrored by PE
       transpose (regular matmuls against identity — keeps the HAM
       clock gate seeing a busy PE).
  mm2  computed TRANSPOSED in fp8 DoubleRow (2x PE throughput):
           outT[hw, c] = qT @ (gamma*attn^T/Z) + qT
       stationary = qn8 (host-cast fp8 q, channel-pair layout
       [t2, P, 2, hw]), moving = attnT8 (PE-transposed softmax rows,
       cast fp32->fp8 on the PSUM->SBUF copyback). 2 DoubleRow matmuls
       per 128x512 group instead of 4 fp16 ones. Emitted in PAIRS of hw
       blocks ([P, 2, C] 2-bank PSUM tiles) so copy/add instruction
       overheads amortize over 1024 elements.
  residual  the +x term must NOT pass through fp8 (3% noise on x blows
       the error budget), so it is NOT identity-folded in fp8: the fp16
       qt tile (loaded for mm1 anyway) is added on the way out, via a
       per-pair route mix (ACT copy + DVE/GPSIMD add, fused DVE
       PSUM-add, or a PE fp16-identity matmul) chosen so every engine
       stays under the PE pair rate in each phase. Output is fp16 outT
       in pair-blocked layout; the host re-indexes and casts fp32.
       Removes the qn fp16 load entirely (DMA 21MB/core vs v7 33.6MB).

Scheduling (PE is in-order; emission order = schedule):
  - warmups read a memset tile (no DMA dependency) so the PE is busy
    from the end of the ~7us engine preamble and the p-state ramp is
    done before mm1; sized to bridge until qt0's transfer lands.
  - mm1(b1) is row-major filler inside softmax(0); rows open in pairs
    {0,1}/{2,3} gated on the odd exp of batch 0, because mm1 rows live
    in HALVES of [P, 2, C] psim pair-buffers — which later serve as
    full 2-bank mm2 pairs, giving the drain a 4-pair PSUM rotation.
  - mm2(b0) pairs are filler inside softmax(1); mm2(b1) drains after.
  - transposed-p accumulators (pst) also live as pair-buffer halves and
    take transpose blocks per-ci right after each p_s, so only one
    ci-column + the fp8 casts (split ACT/DVE) remain after the last p_s.
  - stores: one 256KB DMA per pair, issued with a 2-pair lag (an ACT/
    sync-queue dma_start must never wait on an unfinished add), on the
    sync + scalar HARDWARE DGE rings only; gpsimd's ring is software
    DGE (~2x slower — it caused a 5us end-of-kernel drain and an
    intermittent corruption when mixed with GPSIMD compute).

  Measured numerics (host sim matches hw exactly): rel_l2 1.15e-2 vs
  the 2e-2 gate (fp8 rounding of attn and q contribute ~0.8e-2 each).
"""
import sys

if "/opt/trn_rl_repo" not in sys.path:
    sys.path.insert(0, "/opt/trn_rl_repo")

import numpy as np
import ml_dtypes

B, C, H, W = 16, 512, 64, 64
HW = H * W
NCORES = 8
NB = B // NCORES          # batches per core
P = 128
CB = C // P               # 4 channel blocks
KN = HW // P              # 32 contraction chunks for sim
KT = KN // 4              # 8 transposed-q tiles of 4 chunks each
T2 = C // 256             # 2 channel double-tiles for DoubleRow mm2
NPAIR = HW // P // 2      # 16 mm2 output pairs (of 128-row hw blocks)

_BUILD_CACHE = {}


def build_bass():
    import concourse.bacc as bacc
    import concourse.tile as tile
    from concourse import mybir

    f32 = mybir.dt.float32
    f16 = mybir.dt.float16
    f8 = mybir.dt.float8e4
    AX = mybir.AxisListType
    ALU = mybir.AluOpType
    ACTF = mybir.ActivationFunctionType
    DR = mybir.MatmulPerfMode.DoubleRow

    nc = bacc.Bacc()
    # qt: q transposed, host-arranged as [KT, P, 4, C] so one DMA per
    # 4-chunk tile is fully contiguous (4KB descriptor lines). Feeds mm1
    # AND is the fp16 residual source for the transposed mm2.
    qt_ext = nc.declare_dram_parameter("qt", [NB, KT, P, 4, C], f16,
                                       isOutput=False)
    # qn8: fp8 q in channel-pair layout for DoubleRow stationary tiles:
    # qn8[b, t2, p, i, hw] = fp8(q[ch = t2*256 + i*128 + p, hw])
    q8_ext = nc.declare_dram_parameter("qn8", [NB, T2, P, 2, HW], f8,
                                       isOutput=False)
    g_ext = nc.declare_dram_parameter("gamma", [1], f32, isOutput=False)
    i_ext = nc.declare_dram_parameter("ident", [P, P], f16, isOutput=False)
    # outT stored in pair-blocked layout [b, g, p, j, c] = out^T row
    # hw=(2g+j)*128+p: one 256KB DMA per pair with contiguous 2KB runs;
    # the host re-indexes (transpose(0,1,3,2,4)) for free
    o_ext = nc.declare_dram_parameter("outT", [NB, NPAIR, P, 2, C], f16,
                                      isOutput=True)

    _flip = [0]

    with tile.TileContext(nc) as tc:
        with (
            tc.tile_pool(name="const", bufs=1) as const,
            tc.tile_pool(name="qt", bufs=16) as qtp,
            tc.tile_pool(name="qn8", bufs=4) as qn8p,
            tc.tile_pool(name="a8", bufs=4) as a8p,
            tc.tile_pool(name="pp", bufs=4) as pp,
            tc.tile_pool(name="fsb", bufs=8) as fsbp,
            tc.tile_pool(name="stg", bufs=10) as stgp,
            tc.tile_pool(name="tri", bufs=6) as trip,
            tc.tile_pool(name="vec", bufs=6) as vec,
            tc.tile_pool(name="psim", bufs=2, space="PSUM") as psimp,
            tc.tile_pool(name="ppair", bufs=2, space="PSUM") as ppairp,
        ):
            def copyback(dst, src):
                if _flip[0] % 2 == 0:
                    nc.scalar.copy(dst, src)
                else:
                    nc.vector.tensor_copy(dst, src)
                _flip[0] += 1

            # ident+gamma on the scalar HW ring so the sync ring's first
            # transfer is the critical qt0 tile
            ident_h = const.tile([P, P], f16)
            nc.scalar.dma_start(out=ident_h[:], in_=i_ext[:])
            gamma_sb = const.tile([P, 1], f32)
            nc.scalar.dma_start(out=gamma_sb[:], in_=g_ext[:].to_broadcast([P, 1]))

            # data loads all on the sync ring in need-order (one ring's
            # queued transfers already saturate HBM; the first-transfer
            # delay is completion latency, not bandwidth — splitting
            # loads across rings only delays the first tile)
            def load_qt(b, st, t):
                qt4 = qtp.tile([P, 4, C], f16, tag="qt", name=f"qt{b}_{t}")
                nc.sync.dma_start(out=qt4[:, :, :], in_=qt_ext[b, t])
                st["qt"][t] = qt4

            def load_qn8(b, st, t2):
                q8 = qn8p.tile([P, 2, HW], f8, tag="qn8", name=f"q8{b}_{t2}")
                nc.sync.dma_start(out=q8[:, :, :], in_=q8_ext[b, t2])
                st["qn8"][t2] = q8

            def alloc_state(b):
                return {"qt": {}, "qn8": {}, "psim": {}, "a8": {}}

            def mm_transpose(out, in_):
                nc.tensor.matmul(out, in_, ident_h[:], start=True, stop=True)

            # real warmup matmuls while the first loads land (HAM warm-up;
            # sized to end right as qt0's transfer completes, so the
            # p-state ramp doesn't reset and mm1 starts at full clock).
            # They read an UNINITIALIZED SBUF tile — no DMA dependency, so
            # they start at ~6us (right after the engine preamble) instead
            # of waiting ~4us for ident's first-DMA completion latency;
            # the garbage results land in a psum tile nobody reads.
            warm_src = const.tile([P, P], f16)
            nc.gpsimd.memset(warm_src[:], 0)
            warm = ppairp.tile([P, 2, C], f32, tag="pp", name="warmup")
            for i in range(64):
                nc.tensor.matmul(warm[:, 0, :P], warm_src[:], warm_src[:],
                                 start=True, stop=True)

            C0S = [mi * P for mi in range(CB)]  # exact upper triangle
            TRI = {1: [(1, 0)], 2: [(2, 0), (2, 1)], 3: [(3, 0), (3, 1), (3, 2)]}

            def alloc_psim_pair(st, mi, b):
                """mm1 rows live in halves of [P, 2, C] pair-buffers so the
                mm2 drain can reuse the psim ring as full 2-bank pairs."""
                buf = psimp.tile([P, 2, C], f32, tag="psim",
                                 name=f"psim{b}_{mi}")
                st["psim"][mi] = buf[:, 0, :]
                st["psim"][mi + 1] = buf[:, 1, :]

            def mm1_group(st, mi, kn, b):
                t, kq = divmod(kn, 4)
                c0 = C0S[mi]
                qt4 = st["qt"][t]
                nc.tensor.matmul(
                    st["psim"][mi][:, c0:],
                    qt4[:, kq, mi * P:(mi + 1) * P],
                    qt4[:, kq, c0:],
                    start=(kn == 0),
                    stop=(kn == KN - 1),
                )

            def phase1_b0(st):
                """batch 0: kn-major (starts on the first loaded tile)."""
                for mi in range(0, CB, 2):
                    alloc_psim_pair(st, mi, 0)
                for kn in range(KN):
                    for mi in range(CB):
                        mm1_group(st, mi, kn, 0)

            def mm1_b1_filler(st):
                """batch 1: row-major generator — rows open in pairs {0,1}
                then {2,3}, enabled by the caller right after the odd exp
                of batch 0 frees that psim pair-buffer."""
                state = {"mi": 0, "kn": 0, "maxmi": 0}

                def allow(mi):
                    state["maxmi"] = max(state["maxmi"], mi + 1)

                def emit(n):
                    for _ in range(n):
                        mi, kn = state["mi"], state["kn"]
                        if mi >= CB:
                            return
                        if mi >= state["maxmi"]:
                            return
                        if kn == 0 and mi % 2 == 0:
                            alloc_psim_pair(st, mi, 1)
                        mm1_group(st, mi, kn, 1)
                        if kn == KN - 1:
                            state["mi"], state["kn"] = mi + 1, 0
                        else:
                            state["kn"] = kn + 1

                def flush():
                    state["maxmi"] = CB
                    emit(CB * KN)

                return allow, emit, flush

            def softmax_a8(b, st, on_exp=None, filler=None, head_dummy=False,
                           pst_pool_tag=None):
                """tri fills + rowwise softmax (pipelined per block-row),
                then attnT8 = fp8(T(p*gamma/Z)) in channel-pair layout.
                `filler(n)` emits ready next-phase matmuls between stages
                (PE is in-order); `on_exp(mi)` notifies psim[mi] consumed.
                pst transposes borrow whichever PSUM ring is idle during
                this batch's softmax (ppair in S0, psim in S1)."""
                psim = st["psim"]
                pst_pool, pst_tag = pst_pool_tag
                # cover the serial tri-cast/reduce/exp head: real filler
                # groups if available, else dummy matmuls on loaded data
                if filler is not None and not head_dummy:
                    filler(4)
                # cover the serial tri-cast/reduce/exp head with dummy
                # matmuls on loaded data (real next-phase work is gated).
                # NOTE: keep the PE queue gap-free here — a data-gated PE
                # instruction between mm1 and the dummies can bubble and
                # trigger a HAM downclock of the whole kernel (~+25us).
                if head_dummy:
                    dmy = ppairp.tile([P, 2, C], f32, tag="pp", name=f"dmy{b}")
                    src = st["qt"][KT - 1]
                    for i in range(16):
                        nc.tensor.matmul(dmy[:, 0, :], ident_h[:],
                                         src[:, i % 4, :],
                                         start=True, stop=True)
                # row 0 needs no tri mirror: launch its reduce+exp first so
                # the ACT/DVE pipelines fill immediately
                def reduce_exp(mi):
                    mrow = vec.tile([P, 1], f32, tag="mrow")
                    nc.vector.tensor_reduce(
                        mrow[:], psim[mi][:], axis=AX.X, op=ALU.min
                    )
                    zrow = vec.tile([P, 1], f32, tag="zrow")
                    p_t = pp.tile([P, C], f16, tag="p", bufs=4)
                    nc.scalar.activation(
                        p_t[:], psim[mi][:], ACTF.Exp,
                        bias=mrow[:], scale=-1.0, accum_out=zrow[:],
                    )
                    zrows.append(zrow)
                    p_ts.append(p_t)
                    if on_exp is not None:
                        on_exp(mi)
                    if filler is not None:
                        filler(5)

                zrows, p_ts = [], []
                reduce_exp(0)
                # ALL tri tmp copybacks (the psim[j] READERS) must be
                # emitted before the filler may allocate the next batch's
                # psim buffers (gated on exp(1)/exp(3)), else the pool
                # rotation races the still-unemitted mirror reads.
                tmps = {}
                for mi in range(CB):
                    for (i, j) in TRI.get(mi, []):
                        tmp = trip.tile([P, P], f16, tag="tri")
                        copyback(tmp[:], psim[j][:, i * P:(i + 1) * P])
                        tmps[(i, j)] = tmp
                # per-row: mirror transposes then reduce+exp
                for mi in range(1, CB):
                    for (i, j) in TRI.get(mi, []):
                        mm_transpose(psim[i][:, j * P:(j + 1) * P],
                                     tmps[(i, j)][:])
                    reduce_exp(mi)
                # all 4 pst (transposed-p) accumulators live as halves of
                # two [P, 2, C] pair-buffers (4 banks), so transpose blocks
                # can be emitted per-ci RIGHT AFTER each p_s(ci) — after the
                # last p_s only one ci-column of transposes + the casts
                # remain, instead of the whole 16-transpose chain
                pstA = pst_pool.tile([P, 2, C], f32, tag=pst_tag,
                                     name=f"pstA{b}")
                pstB = pst_pool.tile([P, 2, C], f32, tag=pst_tag,
                                     name=f"pstB{b}")
                psts = [pstA[:, 0, :], pstA[:, 1, :],
                        pstB[:, 0, :], pstB[:, 1, :]]
                # stage B: per-row 1/Z, gamma fold, fp16 scale, transposes
                for mi in range(CB):
                    rz = vec.tile([P, 1], f32, tag="rz")
                    nc.vector.reciprocal(rz[:], zrows[mi][:])
                    rzg = vec.tile([P, 1], f32, tag="rzg")
                    nc.vector.tensor_mul(rzg[:], rz[:], gamma_sb[:])
                    p_s = pp.tile([P, C], f16, tag="psc", bufs=4)
                    nc.vector.tensor_scalar_mul(p_s[:], p_ts[mi][:], rzg[:])
                    if filler is not None:
                        filler(2)
                    for kd in range(CB):
                        mm_transpose(
                            psts[kd][:, mi * P:(mi + 1) * P],
                            p_s[:, kd * P:(kd + 1) * P],
                        )
                    if filler is not None:
                        filler(3)
                # cast fp8 into the channel-pair moving tiles
                # a8[t2][:, i, :], kd = 2*t2 + i; casts alternate ACT/DVE
                # so the tail runs 2 in parallel
                a8t = None
                for kd in range(CB):
                    t2, i = divmod(kd, 2)
                    if i == 0:
                        a8t = a8p.tile([P, 2, C], f8, tag="a8",
                                       name=f"a8_{b}_{t2}")
                        st["a8"][t2] = a8t
                    # no filler here: these casts gate the next mm2 phase,
                    # and filler copies/adds would queue ahead of them on
                    # the in-order ACT/DVE queues
                    if kd % 2 == 0:
                        nc.scalar.copy(a8t[:, i, :], psts[kd][:])
                    else:
                        nc.vector.tensor_copy(a8t[:, i, :], psts[kd][:])

            def mm2_emitter(b, st, rings, pools, mode):
                """outT[hw, :] = qT@attnT8*gamma/Z + qT, emitted in PAIRS of
                128-row hw blocks: one [P, 2, C] PSUM tile (2 banks) per
                pair. emit(n) is PE filler inside the next batch's softmax;
                n counts pairs.

                Residual routes per pair, mixed so every engine stays under
                the PE pair rate (DR pair = 864ns, +426ns if the residual
                is PE-identity-folded):
                  peresid: +2 fp16 identity matmuls, plain ACT copyback
                  direct:  one fused DVE add straight from PSUM (1209ns)
                  actgps:  ACT copy -> fp16 SBUF, GPSIMD add (GPSIMD's
                           compute FIFOs are free; only its DMA ring is
                           software-DGE-slow)
                  actdve:  ACT copy -> fp16 SBUF, DVE add (692ns, 2x rate)
                Stores are issued with a 2-pair lag so a store on the ACT
                queue never waits on an unfinished add, and go to the sync
                HW ring (idle in these phases); the last drain pairs use
                the scalar HW ring for a short final flush."""
                qn8_t, a8, qt_t = st["qn8"], st["a8"], st["qt"]
                state = {"g": 0, "pending": []}

                def ring_for(g):
                    if mode == "s1":
                        return nc.sync
                    return nc.sync if g < NPAIR - 2 else nc.scalar

                def issue_store(g, stg):
                    ring_for(g).dma_start(out=o_ext[b, g], in_=stg[:])

                def emit(n):
                    for _ in range(n):
                        g = state["g"]
                        if g >= NPAIR:
                            # terminal flush of the lagged stores
                            for gq, stgq in state["pending"]:
                                issue_store(gq, stgq)
                            state["pending"] = []
                            return
                        if mode == "s1":
                            r = g % 8
                            if g >= NPAIR - 3:
                                # last s1 pairs overlap the softmax tail
                                # whose casts gate the drain: keep their
                                # engine footprint off the ACT/DVE queues
                                route = "peresid"
                            else:
                                route = ("peresid" if r == 0 else
                                         "direct" if r % 2 == 1 else "actgps")
                        elif g >= NPAIR - 3:
                            # no slow GPSIMD adds near the tail
                            route = "direct" if g % 2 == 1 else "actdve"
                        else:
                            route = ("direct" if g % 2 == 1 else
                                     "actgps" if g % 4 == 0 else "actdve")
                        pool, ptag = pools[g % len(pools)]
                        pf = pool.tile([P, 2, C], f32, tag=ptag,
                                       name=f"pf{b}_{g}")
                        m0 = 2 * g
                        t, kq = divmod(m0, 4)
                        for j in range(2):
                            m = 2 * g + j
                            for t2 in range(T2):
                                nc.tensor.matmul(
                                    pf[:, j, :],
                                    qn8_t[t2][:, :, m * P:(m + 1) * P],
                                    a8[t2][:, :, :],
                                    start=(t2 == 0),
                                    stop=(t2 == T2 - 1 and route != "peresid"),
                                    perf_mode=DR,
                                )
                            if route == "peresid":
                                # residual: out^T block += I @ qT chunk
                                nc.tensor.matmul(
                                    pf[:, j, :],
                                    ident_h[:],
                                    qt_t[t][:, kq + j, :],
                                    start=False,
                                    stop=True,
                                )
                        stg = stgp.tile([P, 2, C], f16, tag="stg")
                        if route == "peresid":
                            if g % 2 == 0:
                                nc.scalar.copy(stg[:], pf[:])
                            else:
                                nc.vector.tensor_copy(stg[:], pf[:])
                        elif route == "direct":
                            nc.vector.tensor_add(stg[:], pf[:],
                                                 qt_t[t][:, kq:kq + 2, :])
                        else:
                            fsb = fsbp.tile([P, 2, C], f16, tag="fsb")
                            nc.scalar.copy(fsb[:], pf[:])
                            eng = nc.gpsimd if route == "actgps" else nc.vector
                            eng.tensor_add(stg[:], fsb[:],
                                           qt_t[t][:, kq:kq + 2, :])
                        state["pending"].append((g, stg))
                        if len(state["pending"]) > 2:
                            gq, stgq = state["pending"].pop(0)
                            issue_store(gq, stgq)
                        state["g"] = g + 1

                return emit

            # ---- emission ----
            # keep the number of in-flight DMAs at startup small (8 DMAHW
            # semaphore lanes): ident + qt(b0) first, everything else after
            st0 = alloc_state(0)
            st1 = alloc_state(1)
            for t in range(KT):
                load_qt(0, st0, t)
            phase1_b0(st0)
            # need-order on the single load ring: qt1 feeds the mm1(1)
            # filler; qn8 is only needed by mm2
            for t in range(KT):
                load_qt(1, st1, t)
            for t2 in range(T2):
                load_qn8(0, st0, t2)
            for t2 in range(T2):
                load_qn8(1, st1, t2)
            allow, emit_mm1, flush_mm1 = mm1_b1_filler(st1)

            def sm0_on_exp(mi):
                # b1's psim pair-buffer {mi-1, mi} only frees at the odd exp
                if mi % 2 == 1:
                    allow(mi)

            softmax_a8(0, st0, on_exp=sm0_on_exp, filler=emit_mm1,
                       head_dummy=True, pst_pool_tag=(ppairp, "pp"))
            flush_mm1()
            rings = [nc.sync, nc.scalar]
            mm2_0 = mm2_emitter(0, st0, rings, pools=[(ppairp, "pp")],
                                mode="s1")
            softmax_a8(1, st1, filler=mm2_0, pst_pool_tag=(psimp, "psim"))
            mm2_0(NPAIR + 1)  # flush remaining pairs + lagged stores
            # drain: psim ring is free now — rotate pairs across both rings
            # (4 pairs in flight) so PE never waits on a copyback
            mm2_1 = mm2_emitter(1, st1, rings,
                                pools=[(ppairp, "pp"), (psimp, "psim")],
                                mode="mix")
            mm2_1(NPAIR + 1)

    nc.finalize()
    return nc


def get_bass():
    if "nc" not in _BUILD_CACHE:
        _BUILD_CACHE["nc"] = build_bass()
    return _BUILD_CACHE["nc"]


_IDENT = None


def make_in_maps(x, gamma):
    global _IDENT
    if _IDENT is None:
        _IDENT = np.eye(P, dtype=np.float16)
    x = np.asarray(x, dtype=np.float32).reshape(B, C, HW)
    qn16 = x.astype(np.float16)
    # [B, KT, P, 4, C]: within each 4-chunk tile, partition-major so the
    # device DMA is fully contiguous (4KB descriptor lines)
    qt = np.ascontiguousarray(
        qn16.transpose(0, 2, 1)          # [B, HW, C]
        .reshape(B, KT, 4, P, C)         # [B, t, k, p, C]
        .transpose(0, 1, 3, 2, 4)        # [B, t, p, k, C]
    )
    # [B, T2, P, 2, HW] fp8 channel-pair layout (cast from fp32 directly)
    qn8 = np.ascontiguousarray(
        x.reshape(B, T2, 2, P, HW)       # [B, t2, i, p, hw]
        .transpose(0, 1, 3, 2, 4)        # [B, t2, p, i, hw]
    ).astype(ml_dtypes.float8_e4m3)
    gamma = np.asarray(gamma, dtype=np.float32).reshape(1)
    return [
        {
            "qt": qt[i * NB:(i + 1) * NB],
            "qn8": qn8[i * NB:(i + 1) * NB],
            "gamma": gamma,
            "ident": _IDENT,
        }
        for i in range(NCORES)
    ]


def run(x, gamma, trace=False, **trace_kwargs):
    from concourse.bass_utils import run_bass_kernel_spmd

    nc = get_bass()
    res = run_bass_kernel_spmd(
        nc, make_in_maps(x, gamma), core_ids=list(range(NCORES)),
        trace=trace, **trace_kwargs,
    )
    outT = np.concatenate(
        [res.results[i]["outT"] for i in range(NCORES)], axis=0
    )  # [B, NPAIR, P, 2, C] fp16, pair-blocked: row hw = (2g+j)*128+p
    outT = outT.transpose(0, 1, 3, 2, 4).reshape(B, HW, C)
    out = outT.astype(np.float32).transpose(0, 2, 1).reshape(B, C, H, W)
    return np.ascontiguousarray(out), res


def kernel(x, gamma):
    out, _ = run(x, gamma, trace=False)
    return out


# revision 53
# speedup vs baseline: 1.2872x; 1.0247x over previous
"""Channel-attention (CAM) Trainium2 kernel, v8 (final).

Reference computation (per batch b of 16):
    q   = x[b].reshape(C, HW)                  # C=512, HW=4096
    sim = q @ q.T                              # [C, C], symmetric
    attn = softmax(max(sim) - sim, axis=-1)    # == exp(min_r - sim) / Z_r
    out[b] = gamma * attn @ x[b] + x[b]

Sharding: data-parallel over batch across 8 NeuronCores (2 batches/core).
kernel() takes full inputs, shards internally, returns the full output.
Measured ~100us/core vs the 118us fp16 v7 baseline.

Design (on top of v7's interleaved-emission skeleton):

  mm1  sim upper block-triangle in fp16 (sim feeds an exp, so it needs
       fp16 precision; fp8-with-correction needs 3 half-rate products vs
       1 full-rate, strictly worse). Lower blocks mirrored by PE
       transpose (regular matmuls against identity — keeps the HAM
       clock gate seeing a busy PE).
  mm2  computed TRANSPOSED in fp8 DoubleRow (2x PE throughput):
           outT[hw, c] = qT @ (gamma*attn^T/Z) + qT
       stationary = qn8 (host-cast fp8 q, channel-pair layout
       [t2, P, 2, hw]), moving = attnT8 (PE-transposed softmax rows,
       cast fp32->fp8 on the PSUM->SBUF copyback). 2 DoubleRow matmuls
       per 128x512 group instead of 4 fp16 ones. Emitted in PAIRS of hw
       blocks ([P, 2, C] 2-bank PSUM tiles) so copy/add instruction
       overheads amortize over 1024 elements.
  residual  the +x term must NOT pass through fp8 (3% noise on x blows
       the error budget), so it is NOT identity-folded in fp8: the fp16
       qt tile (loaded for mm1 anyway) is added on the way out, via a
       per-pair route mix (ACT copy + DVE/GPSIMD add, fused DVE
       PSUM-add, or a PE fp16-identity matmul) chosen so every engine
       stays under the PE pair rate in each phase. Output is fp16 outT
       in pair-blocked layout; the host re-indexes and casts fp32.
       Removes the qn fp16 load entirely (DMA 21MB/core vs v7 33.6MB).

Scheduling (PE is in-order; emission order = schedule):
  - warmups read a memset tile (no DMA dependency) so the PE is busy
    from the end of the ~7us engine preamble and the p-state ramp is
    done before mm1; sized to bridge until qt0's transfer lands.
  - mm1(b1) is row-major filler inside softmax(0); rows open in pairs
    {0,1}/{2,3} gated on the odd exp of batch 0, because mm1 rows live
    in HALVES of [P, 2, C] psim pair-buffers — which later serve as
    full 2-bank mm2 pairs, giving the drain a 4-pair PSUM rotation.
  - mm2(b0) pairs are filler inside softmax(1); mm2(b1) drains after.
  - transposed-p accumulators (pst) also live as pair-buffer halves and
    take transpose blocks per-ci right after each p_s, so only one
    ci-column + the fp8 casts (split ACT/DVE) remain after the last p_s.
  - stores: one 256KB DMA per pair, issued with a 2-pair lag (an ACT/
    sync-queue dma_start must never wait on an unfinished add), on the
    sync + scalar HARDWARE DGE rings only; gpsimd's ring is software
    DGE (~2x slower — it caused a 5us end-of-kernel drain and an
    intermittent corruption when mixed with GPSIMD compute).

  Measured numerics (host sim matches hw exactly): rel_l2 1.15e-2 vs
  the 2e-2 gate (fp8 rounding of attn and q contribute ~0.8e-2 each).
"""
import sys

if "/opt/trn_rl_repo" not in sys.path:
    sys.path.insert(0, "/opt/trn_rl_repo")

import numpy as np
import ml_dtypes

B, C, H, W = 16, 512, 64, 64
HW = H * W
NCORES = 8
NB = B // NCORES          # batches per core
P = 128
CB = C // P               # 4 channel blocks
KN = HW // P              # 32 contraction chunks for sim
KT = KN // 4              # 8 transposed-q tiles of 4 chunks each
T2 = C // 256             # 2 channel double-tiles for DoubleRow mm2
NPAIR = HW // P // 2      # 16 mm2 output pairs (of 128-row hw blocks)

_BUILD_CACHE = {}


def build_bass():
    import concourse.bacc as bacc
    import concourse.tile as tile
    from concourse import mybir

    f32 = mybir.dt.float32
    f16 = mybir.dt.float16
    f8 = mybir.dt.float8e4
    AX = mybir.AxisListType
    ALU = mybir.AluOpType
    ACTF = mybir.ActivationFunctionType
    DR = mybir.MatmulPerfMode.DoubleRow

    nc = bacc.Bacc()
    # qt: q transposed, host-arranged as [KT, P, 4, C] so one DMA per
    # 4-chunk tile is fully contiguous (4KB descriptor lines). Feeds mm1
    # AND is the fp16 residual source for the transposed mm2.
    qt_ext = nc.declare_dram_parameter("qt", [NB, KT, P, 4, C], f16,
                                       isOutput=False)
    # qn8: fp8 q in channel-pair layout for DoubleRow stationary tiles:
    # qn8[b, t2, p, i, hw] = fp8(q[ch = t2*256 + i*128 + p, hw])
    q8_ext = nc.declare_dram_parameter("qn8", [NB, T2, P, 2, HW], f8,
                                       isOutput=False)
    g_ext = nc.declare_dram_parameter("gamma", [1], f32, isOutput=False)
    i_ext = nc.declare_dram_parameter("ident", [P, P], f16, isOutput=False)
    # outT stored in pair-blocked layout [b, g, p, j, c] = out^T row
    # hw=(2g+j)*128+p: one 256KB DMA per pair with contiguous 2KB runs;
    # the host re-indexes (transpose(0,1,3,2,4)) for free
    o_ext = nc.declare_dram_parameter("outT", [NB, NPAIR, P, 2, C], f16,
                                      isOutput=True)

    _flip = [0]

    with tile.TileContext(nc) as tc:
        with (
            tc.tile_pool(name="const", bufs=1) as const,
            tc.tile_pool(name="qt", bufs=16) as qtp,
            tc.tile_pool(name="qn8", bufs=4) as qn8p,
            tc.tile_pool(name="a8", bufs=4) as a8p,
            tc.tile_pool(name="pp", bufs=4) as pp,
            tc.tile_pool(name="fsb", bufs=8) as fsbp,
            tc.tile_pool(name="stg", bufs=10) as stgp,
            tc.tile_pool(name="tri", bufs=6) as trip,
            tc.tile_pool(name="vec", bufs=6) as vec,
            tc.tile_pool(name="psim", bufs=2, space="PSUM") as psimp,
            tc.tile_pool(name="ppair", bufs=2, space="PSUM") as ppairp,
        ):
            def copyback(dst, src):
                if _flip[0] % 2 == 0:
                    nc.scalar.copy(dst, src)
                else:
                    nc.vector.tensor_copy(dst, src)
                _flip[0] += 1

            # ident+gamma on the scalar HW ring so the sync ring's first
            # transfer is the critical qt0 tile
            ident_h = const.tile([P, P], f16)
            nc.scalar.dma_start(out=ident_h[:], in_=i_ext[:])
            gamma_sb = const.tile([P, 1], f32)
            nc.scalar.dma_start(out=gamma_sb[:], in_=g_ext[:].to_broadcast([P, 1]))

            # data loads all on the sync ring in need-order (one ring's
            # queued transfers already saturate HBM; the first-transfer
            # delay is completion latency, not bandwidth — splitting
            # loads across rings only delays the first tile)
            def load_qt(b, st, t):
                qt4 = qtp.tile([P, 4, C], f16, tag="qt", name=f"qt{b}_{t}")
                nc.sync.dma_start(out=qt4[:, :, :], in_=qt_ext[b, t])
                st["qt"][t] = qt4

            def load_qn8(b, st, t2):
                q8 = qn8p.tile([P, 2, HW], f8, tag="qn8", name=f"q8{b}_{t2}")
                nc.sync.dma_start(out=q8[:, :, :], in_=q8_ext[b, t2])
                st["qn8"][t2] = q8

            def alloc_state(b):
                return {"qt": {}, "qn8": {}, "psim": {}, "a8": {}}

            def mm_transpose(out, in_):
                nc.tensor.matmul(out, in_, ident_h[:], start=True, stop=True)

            # real warmup matmuls while the first loads land (HAM warm-up;
            # sized to end right as qt0's transfer completes, so the
            # p-state ramp doesn't reset and mm1 starts at full clock).
            # They read an UNINITIALIZED SBUF tile — no DMA dependency, so
            # they start at ~6us (right after the engine preamble) instead
            # of waiting ~4us for ident's first-DMA completion latency;
            # the garbage results land in a psum tile nobody reads.
            warm_src = const.tile([P, P], f16)
            nc.gpsimd.memset(warm_src[:], 0)
            warm = ppairp.tile([P, 2, C], f32, tag="pp", name="warmup")
            for i in range(64):
                nc.tensor.matmul(warm[:, 0, :P], warm_src[:], warm_src[:],
                                 start=True, stop=True)

            C0S = [mi * P for mi in range(CB)]  # exact upper triangle
            TRI = {1: [(1, 0)], 2: [(2, 0), (2, 1)], 3: [(3, 0), (3, 1), (3, 2)]}

            def alloc_psim_pair(st, mi, b):
                """mm1 rows live in halves of [P, 2, C] pair-buffers so the
                mm2 drain can reuse the psim ring as full 2-bank pairs."""
                buf = psimp.tile([P, 2, C], f32, tag="psim",
                                 name=f"psim{b}_{mi}")
                st["psim"][mi] = buf[:, 0, :]
                st["psim"][mi + 1] = buf[:, 1, :]

            def mm1_group(st, mi, kn, b):
                t, kq = divmod(kn, 4)
                c0 = C0S[mi]
                qt4 = st["qt"][t]
                nc.tensor.matmul(
                    st["psim"][mi][:, c0:],
                    qt4[:, kq, mi * P:(mi + 1) * P],
                    qt4[:, kq, c0:],
                    start=(kn == 0),
                    stop=(kn == KN - 1),
                )

            def phase1_b0(st):
                """batch 0: kn-major (starts on the first loaded tile)."""
                for mi in range(0, CB, 2):
                    alloc_psim_pair(st, mi, 0)
                for kn in range(KN):
                    for mi in range(CB):
                        mm1_group(st, mi, kn, 0)

            def mm1_b1_filler(st):
                """batch 1: row-major generator — rows open in pairs {0,1}
                then {2,3}, enabled by the caller right after the odd exp
                of batch 0 frees that psim pair-buffer."""
                state = {"mi": 0, "kn": 0, "maxmi": 0}

                def allow(mi):
                    state["maxmi"] = max(state["maxmi"], mi + 1)

                def emit(n):
                    for _ in range(n):
                        mi, kn = state["mi"], state["kn"]
                        if mi >= CB:
                            return
                        if mi >= state["maxmi"]:
                            return
                        if kn == 0 and mi % 2 == 0:
                            alloc_psim_pair(st, mi, 1)
                        mm1_group(st, mi, kn, 1)
                        if kn == KN - 1:
                            state["mi"], state["kn"] = mi + 1, 0
                        else:
                            state["kn"] = kn + 1

                def flush():
                    state["maxmi"] = CB
                    emit(CB * KN)

                return allow, emit, flush

            def softmax_a8(b, st, on_exp=None, filler=None, head_dummy=False,
                           pst_pool_tag=None):
                """tri fills + rowwise softmax (pipelined per block-row),
                then attnT8 = fp8(T(p*gamma/Z)) in channel-pair layout.
                `filler(n)` emits ready next-phase matmuls between stages
                (PE is in-order); `on_exp(mi)` notifies psim[mi] consumed.
                pst transposes borrow whichever PSUM ring is idle during
                this batch's softmax (ppair in S0, psim in S1)."""
                psim = st["psim"]
                pst_pool, pst_tag = pst_pool_tag
                # cover the serial tri-cast/reduce/exp head: real filler
                # groups if available, else dummy matmuls on loaded data
                if filler is not None and not head_dummy:
                    filler(4)
                # cover the serial tri-cast/reduce/exp head with dummy
                # matmuls on loaded data (real next-phase work is gated).
                # NOTE: keep the PE queue gap-free here — a data-gated PE
                # instruction between mm1 and the dummies can bubble and
                # trigger a HAM downclock of the whole kernel (~+25us).
                if head_dummy:
                    dmy = ppairp.tile([P, 2, C], f32, tag="pp", name=f"dmy{b}")
                    src = st["qt"][KT - 1]
                    for i in range(16):
                        nc.tensor.matmul(dmy[:, 0, :], ident_h[:],
                                         src[:, i % 4, :],
                                         start=True, stop=True)
                # row 0 needs no tri mirror: launch its reduce+exp first so
                # the ACT/DVE pipelines fill immediately
                def reduce_exp(mi):
                    mrow = vec.tile([P, 1], f32, tag="mrow")
                    nc.vector.tensor_reduce(
                        mrow[:], psim[mi][:], axis=AX.X, op=ALU.min
                    )
                    zrow = vec.tile([P, 1], f32, tag="zrow")
                    p_t = pp.tile([P, C], f16, tag="p", bufs=4)
                    nc.scalar.activation(
                        p_t[:], psim[mi][:], ACTF.Exp,
                        bias=mrow[:], scale=-1.0, accum_out=zrow[:],
                    )
                    zrows.append(zrow)
                    p_ts.append(p_t)
                    if on_exp is not None:
                        on_exp(mi)
                    if filler is not None:
                        filler(5)

                zrows, p_ts = [], []
                reduce_exp(0)
                # ALL tri tmp copybacks (the psim[j] READERS) must be
                # emitted before the filler may allocate the next batch's
                # psim buffers (gated on exp(1)/exp(3)), else the pool
                # rotation races the still-unemitted mirror reads.
                tmps = {}
                for mi in range(CB):
                    for (i, j) in TRI.get(mi, []):
                        tmp = trip.tile([P, P], f16, tag="tri")
                        copyback(tmp[:], psim[j][:, i * P:(i + 1) * P])
                        tmps[(i, j)] = tmp
                # per-row: mirror transposes then reduce+exp
                for mi in range(1, CB):
                    for (i, j) in TRI.get(mi, []):
                        mm_transpose(psim[i][:, j * P:(j + 1) * P],
                                     tmps[(i, j)][:])
                    reduce_exp(mi)
                # all 4 pst (transposed-p) accumulators live as halves of
                # two [P, 2, C] pair-buffers (4 banks), so transpose blocks
                # can be emitted per-ci RIGHT AFTER each p_s(ci) — after the
                # last p_s only one ci-column of transposes + the casts
                # remain, instead of the whole 16-transpose chain
                pstA = pst_pool.tile([P, 2, C], f32, tag=pst_tag,
                                     name=f"pstA{b}")
                pstB = pst_pool.tile([P, 2, C], f32, tag=pst_tag,
                                     name=f"pstB{b}")
                psts = [pstA[:, 0, :], pstA[:, 1, :],
                        pstB[:, 0, :], pstB[:, 1, :]]
                # stage B: per-row 1/Z, gamma fold, fp16 scale, transposes
                for mi in range(CB):
                    rz = vec.tile([P, 1], f32, tag="rz")
                    nc.vector.reciprocal(rz[:], zrows[mi][:])
                    rzg = vec.tile([P, 1], f32, tag="rzg")
                    nc.vector.tensor_mul(rzg[:], rz[:], gamma_sb[:])
                    p_s = pp.tile([P, C], f16, tag="psc", bufs=4)
                    nc.vector.tensor_scalar_mul(p_s[:], p_ts[mi][:], rzg[:])
                    if filler is not None:
                        filler(2)
                    for kd in range(CB):
                        mm_transpose(
                            psts[kd][:, mi * P:(mi + 1) * P],
                            p_s[:, kd * P:(kd + 1) * P],
                        )
                    if filler is not None:
                        filler(3)
                # cast fp8 into the channel-pair moving tiles
                # a8[t2][:, i, :], kd = 2*t2 + i; casts alternate ACT/DVE
                # so the tail runs 2 in parallel
                a8t = None
                for kd in range(CB):
                    t2, i = divmod(kd, 2)
                    if i == 0:
                        a8t = a8p.tile([P, 2, C], f8, tag="a8",
                                       name=f"a8_{b}_{t2}")
                        st["a8"][t2] = a8t
                    # no filler here: these casts gate the next mm2 phase,
                    # and filler copies/adds would queue ahead of them on
                    # the in-order ACT/DVE queues
                    if kd % 2 == 0:
                        nc.scalar.copy(a8t[:, i, :], psts[kd][:])
                    else:
                        nc.vector.tensor_copy(a8t[:, i, :], psts[kd][:])

            def mm2_emitter(b, st, rings, pools, mode):
                """outT[hw, :] = qT@attnT8*gamma/Z + qT, emitted in PAIRS of
                128-row hw blocks: one [P, 2, C] PSUM tile (2 banks) per
                pair. emit(n) is PE filler inside the next batch's softmax;
                n counts pairs.

                Residual routes per pair, mixed so every engine stays under
                the PE pair rate (DR pair = 864ns, +426ns if the residual
                is PE-identity-folded):
                  peresid: +2 fp16 identity matmuls, plain ACT copyback
                  direct:  one fused DVE add straight from PSUM (1209ns)
                  actgps:  ACT copy -> fp16 SBUF, GPSIMD add (GPSIMD's
                           compute FIFOs are free; only its DMA ring is
                           software-DGE-slow)
                  actdve:  ACT copy -> fp16 SBUF, DVE add (692ns, 2x rate)
                Stores are issued with a 2-pair lag so a store on the ACT
                queue never waits on an unfinished add, and go to the sync
                HW ring (idle in these phases); the last drain pairs use
                the scalar HW ring for a short final flush."""
                qn8_t, a8, qt_t = st["qn8"], st["a8"], st["qt"]
                state = {"g": 0, "pending": []}

                def ring_for(g):
                    if mode == "s1":
                        return nc.sync
                    return nc.sync if g < NPAIR - 2 else nc.scalar

                def issue_store(g, stg):
                    ring_for(g).dma_start(out=o_ext[b, g], in_=stg[:])

                def emit(n):
                    for _ in range(n):
                        g = state["g"]
                        if g >= NPAIR:
                            # terminal flush of the lagged stores
                            for gq, stgq in state["pending"]:
                                issue_store(gq, stgq)
                            state["pending"] = []
                            return
                        if mode == "s1":
                            r = g % 8
                            if g >= NPAIR - 3:
                                # last s1 pairs overlap the softmax tail
                                # whose casts gate the drain: keep their
                                # engine footprint off the ACT/DVE queues
                                route = "peresid"
                            else:
                                route = ("peresid" if r == 0 else
                                         "direct" if r % 2 == 1 else "actgps")
                        elif g >= NPAIR - 3:
                            # no slow GPSIMD adds near the tail
                            route = "direct" if g % 2 == 1 else "actdve"
                        else:
                            route = ("direct" if g % 2 == 1 else
                                     "actgps" if g % 4 == 0 else "actdve")
                        pool, ptag = pools[g % len(pools)]
                        pf = pool.tile([P, 2, C], f32, tag=ptag,
                                       name=f"pf{b}_{g}")
                        m0 = 2 * g
                        t, kq = divmod(m0, 4)
                        for j in range(2):
                            m = 2 * g + j
                            for t2 in range(T2):
                                nc.tensor.matmul(
                                    pf[:, j, :],
                                    qn8_t[t2][:, :, m * P:(m + 1) * P],
                                    a8[t2][:, :, :],
                                    start=(t2 == 0),
                                    stop=(t2 == T2 - 1 and route != "peresid"),
                                    perf_mode=DR,
                                )
                            if route == "peresid":
                                # residual: out^T block += I @ qT chunk
                                nc.tensor.matmul(
                                    pf[:, j, :],
                                    ident_h[:],
                                    qt_t[t][:, kq + j, :],
                                    start=False,
                                    stop=True,
                                )
                        stg = stgp.tile([P, 2, C], f16, tag="stg")
                        if route == "peresid":
                            if g % 2 == 0:
                                nc.scalar.copy(stg[:], pf[:])
                            else:
                                nc.vector.tensor_copy(stg[:], pf[:])
                        elif route == "direct":
                            nc.vector.tensor_add(stg[:], pf[:],
                                                 qt_t[t][:, kq:kq + 2, :])
                        else:
                            fsb = fsbp.tile([P, 2, C], f16, tag="fsb")
                            nc.scalar.copy(fsb[:], pf[:])
                            eng = nc.gpsimd if route == "actgps" else nc.vector
                            eng.tensor_add(stg[:], fsb[:],
                                           qt_t[t][:, kq:kq + 2, :])
                        # sync-queue stores issue immediately (an in-queue
                        # wait on the add's semaphore is harmless there);
                        # only scalar/ACT-queue stores take the 2-pair lag
                        if ring_for(g) is nc.scalar:
                            state["pending"].append((g, stg))
                            if len(state["pending"]) > 2:
                                gq, stgq = state["pending"].pop(0)
                                issue_store(gq, stgq)
                        else:
                            issue_store(g, stg)
                        state["g"] = g + 1

                return emit

            # ---- emission ----
            # keep the number of in-flight DMAs at startup small (8 DMAHW
            # semaphore lanes): ident + qt(b0) first, everything else after
            st0 = alloc_state(0)
            st1 = alloc_state(1)
            for t in range(KT):
                load_qt(0, st0, t)
            phase1_b0(st0)
            # need-order on the single load ring: qt1 feeds the mm1(1)
            # filler; qn8 is only needed by mm2
            for t in range(KT):
                load_qt(1, st1, t)
            for t2 in range(T2):
                load_qn8(0, st0, t2)
            for t2 in range(T2):
                load_qn8(1, st1, t2)
            allow, emit_mm1, flush_mm1 = mm1_b1_filler(st1)

            def sm0_on_exp(mi):
                # b1's psim pair-buffer {mi-1, mi} only frees at the odd exp
                if mi % 2 == 1:
                    allow(mi)

            softmax_a8(0, st0, on_exp=sm0_on_exp, filler=emit_mm1,
                       head_dummy=True, pst_pool_tag=(ppairp, "pp"))
            flush_mm1()
            rings = [nc.sync, nc.scalar]
            mm2_0 = mm2_emitter(0, st0, rings, pools=[(ppairp, "pp")],
                                mode="s1")
            softmax_a8(1, st1, filler=mm2_0, pst_pool_tag=(psimp, "psim"))
            mm2_0(NPAIR + 1)  # flush remaining pairs + lagged stores
            # drain: psim ring is free now — rotate pairs across both rings
            # (4 pairs in flight) so PE never waits on a copyback
            mm2_1 = mm2_emitter(1, st1, rings,
                                pools=[(ppairp, "pp"), (psimp, "psim")],
                                mode="mix")
            mm2_1(NPAIR + 1)

    nc.finalize()
    return nc


def get_bass():
    if "nc" not in _BUILD_CACHE:
        _BUILD_CACHE["nc"] = build_bass()
    return _BUILD_CACHE["nc"]


_IDENT = None


def make_in_maps(x, gamma):
    global _IDENT
    if _IDENT is None:
        _IDENT = np.eye(P, dtype=np.float16)
    x = np.asarray(x, dtype=np.float32).reshape(B, C, HW)
    qn16 = x.astype(np.float16)
    # [B, KT, P, 4, C]: within each 4-chunk tile, partition-major so the
    # device DMA is fully contiguous (4KB descriptor lines)
    qt = np.ascontiguousarray(
        qn16.transpose(0, 2, 1)          # [B, HW, C]
        .reshape(B, KT, 4, P, C)         # [B, t, k, p, C]
        .transpose(0, 1, 3, 2, 4)        # [B, t, p, k, C]
    )
    # [B, T2, P, 2, HW] fp8 channel-pair layout (cast from fp32 directly)
    qn8 = np.ascontiguousarray(
        x.reshape(B, T2, 2, P, HW)       # [B, t2, i, p, hw]
        .transpose(0, 1, 3, 2, 4)        # [B, t2, p, i, hw]
    ).astype(ml_dtypes.float8_e4m3)
    gamma = np.asarray(gamma, dtype=np.float32).reshape(1)
    return [
        {
            "qt": qt[i * NB:(i + 1) * NB],
            "qn8": qn8[i * NB:(i + 1) * NB],
            "gamma": gamma,
            "ident": _IDENT,
        }
        for i in range(NCORES)
    ]


def run(x, gamma, trace=False, **trace_kwargs):
    from concourse.bass_utils import run_bass_kernel_spmd

    nc = get_bass()
    res = run_bass_kernel_spmd(
        nc, make_in_maps(x, gamma), core_ids=list(range(NCORES)),
        trace=trace, **trace_kwargs,
    )
    outT = np.concatenate(
        [res.results[i]["outT"] for i in range(NCORES)], axis=0
    )  # [B, NPAIR, P, 2, C] fp16, pair-blocked: row hw = (2g+j)*128+p
    outT = outT.transpose(0, 1, 3, 2, 4).reshape(B, HW, C)
    out = outT.astype(np.float32).transpose(0, 2, 1).reshape(B, C, H, W)
    return np.ascontiguousarray(out), res


def kernel(x, gamma):
    out, _ = run(x, gamma, trace=False)
    return out
